# revision 4
# baseline (speedup 1.0000x reference)
"""Trainium2 Bass kernel for nn_Net_20761871908940 (CNN + state MLP + GNN dueling head).

Strategy (8 NeuronCores, single SPMD program):
  - All 1000-wide graph layers and fc0 are column-parallel (output-feature
    sharded, 125 features per core for the graph layers, 128 for fc0).
    Activations are kept feature-major ([features_part, nodes_free]); after
    each sharded layer the slices are re-assembled with an 8-core AllGather
    through HBM bounce buffers.
  - The conv stack, state MLP, GAT attention softmax, and the fusion tail are
    tiny and run replicated on every core.
  - Weights are pre-transposed / packed on the host so every DMA is a clean
    contiguous 2D copy and every matmul lhsT has contraction on partitions.

kernel(**inputs) takes the FULL unsharded inputs and returns (value, adv).
"""

import numpy as np

import concourse.bass as bass
import concourse.bacc as bacc
import concourse.mybir as mybir
import concourse.tile as tile
from concourse import bass_utils
from concourse.bass import ts

F32 = mybir.dt.float32
AF = mybir.ActivationFunctionType
AX = mybir.AxisListType
ALU = mybir.AluOpType

NCORE = 8
RG = [list(range(NCORE))]
FS = 125  # graph feature slice per core (1000/8)
OS = 128  # fc0 output slice per core (1024/8)

# ---- graph structure (fixed in the model) ----
N = 19
SRC = np.concatenate([np.arange(19), np.arange(1, 18)]).astype(np.int64)
DST = np.concatenate([np.arange(1, 19), np.array([17]), np.arange(17)]).astype(np.int64)
SL_SRC = np.concatenate([SRC, np.arange(19)])
SL_DST = np.concatenate([DST, np.arange(19)])
DEG = 3  # max in-degree with self loops


def _graph_consts():
    nbrs = [[] for _ in range(N)]
    for s, d in zip(SL_SRC, SL_DST):
        nbrs[int(d)].append(int(s))
    sg = np.zeros((N, DEG), np.int64)
    valid = np.zeros((N, DEG), bool)
    for d in range(N):
        assert len(nbrs[d]) <= DEG
        for j, s in enumerate(nbrs[d]):
            sg[d, j] = s
            valid[d, j] = True
    gsT = np.zeros((N, DEG * N), np.float32)  # chunk j: [i, d] = [sg[d,j] == i]
    pj = np.zeros((N, DEG * N), np.float32)  # chunk j: [d, s] = [sg[d,j] == s]
    for d in range(N):
        for j in range(DEG):
            if valid[d, j]:
                gsT[sg[d, j], N * j + d] = 1.0
                pj[d, N * j + sg[d, j]] = 1.0
    padm = np.where(valid, 0.0, -1e30).astype(np.float32)
    # GCN (improved=True) normalized adjacency, transposed
    ew = np.concatenate([np.ones(len(SRC)), 2.0 * np.ones(N)])
    deg = np.zeros(N)
    np.add.at(deg, SL_DST, ew)
    dinv = deg ** -0.5
    norm = dinv[SL_SRC] * ew * dinv[SL_DST]
    agcn = np.zeros((N, N))
    np.add.at(agcn, (SL_DST, SL_SRC), norm)
    agcnT = np.ascontiguousarray(agcn.T).astype(np.float32)
    mesd = np.zeros((2 * NCORE, 2), np.float32)
    mesd[0::2, 0] = 1.0
    mesd[1::2, 1] = 1.0
    return gsT, pj, padm, agcnT, mesd


def _pack_kt(w_slice):
    """w_slice [out_local, 1000] -> SBUF image [125, 8*out_local]:
    SB[p, out_local*j + o] = w_slice[o, 125j + p]."""
    ol = w_slice.shape[0]
    return np.ascontiguousarray(
        w_slice.reshape(ol, 8, FS).transpose(2, 1, 0).reshape(FS, 8 * ol)
    ).astype(np.float32)


def build_program(repeat=1, debug=False):
    nc = bacc.Bacc("TRN2", target_bir_lowering=False, debug=False, num_devices=NCORE)
    names = []

    def din(name, shape):
        names.append(name)
        return nc.dram_tensor(name, list(shape), F32, kind="ExternalInput")

    # ---- sharded inputs (different data per core) ----
    shard_w = {
        k: din(k, [FS, 1000])
        for k in [
            "w_s1l2_l", "w_s1l2_r", "w_gat2", "w_s3l1_l", "w_s3l1_r",
            "w_s3l2_l", "w_s3l2_r", "w_gat4", "w_gcn5",
        ]
    }
    shard_b = {
        k: din(k, [FS, 1])
        for k in ["b_s1l2", "b_gat2", "b_s3l1", "b_s3l2", "b_gat4", "b_gcn5",
                  "as2", "ad2", "as4", "ad4"]
    }
    fc0A_d = din("fc0A", [128, 32 * 128])
    fc0B_d = din("fc0B", [68, 32 * 128])
    fc0b_d = din("fc0b", [128, 1])

    # ---- replicated inputs ----
    s1l1_d = din("s1l1_wlr", [1, 2000])
    b_s1l1_d = din("b_s1l1", [FS, 8])
    p1_d = din("p1", [25, 3721])
    w1T_d = din("w1T", [25, 32])
    b1_d = din("b1", [32, 1])
    w2T_d = din("w2T", [32, 800])
    b2_d = din("b2", [32, 1])
    w3T_d = din("w3T", [32, 800])
    b3_d = din("b3", [32, 1])
    fc1A_d = din("fc1A", [128, 1024])
    fc1B_d = din("fc1B", [128, 1024])
    fc1b_d = din("fc1b", [128, 2])
    fc2T_d = din("fc2T", [4, 64])
    fc2b_d = din("fc2b", [64, 1])
    fc3T_d = din("fc3T", [64, 64])
    fc3b_d = din("fc3b", [64, 1])
    fcgT_d = din("fcgT", [FS, 800])
    fcgb_d = din("fcgb", [100, 1])
    fu_i0_d = din("fu_i0", [128, 256])
    fu_i1_d = din("fu_i1", [128, 256])
    fu_s_d = din("fu_s", [64, 256])
    fu_g_d = din("fu_g", [100, 256])
    fub_d = din("fub", [128, 2])
    headT_d = din("headT", [128, 4])
    headb_d = din("headb", [2, 1])
    ident_d = din("ident", [128, 128])
    gsT_d = din("gsT", [N, DEG * N])
    pj_d = din("pj", [N, DEG * N])
    padm_d = din("padm", [N, DEG])
    mesd_d = din("mesd", [2 * NCORE, 2])
    agcnT_d = din("agcnT", [N, N])
    g0_d = din("g0", [N, 1])
    st_d = din("st", [4, 1])

    out_t = nc.dram_tensor("out", [2, 1], F32, kind="ExternalOutput")
    dbg_t = {}
    if debug:
        for nm, shp in [
            ("dbg_xf1", [1000, 19]), ("dbg_xf2", [1000, 19]), ("dbg_xf3", [1000, 19]),
            ("dbg_xf4", [1000, 19]), ("dbg_xf5", [1000, 19]), ("dbg_xf6", [1000, 19]),
            ("dbg_gmf", [1000, 1]), ("dbg_hf", [1024, 1]),
            ("dbg_img", [128, 2]), ("dbg_s2", [64, 1]), ("dbg_g100", [100, 1]),
            ("dbg_conv1", [32, 3721]), ("dbg_conv2", [32, 841]), ("dbg_conv3", [32, 196]),
            ("dbg_f", [128, 2]),
        ]:
            dbg_t[nm] = nc.dram_tensor(nm, shp, F32, kind="ExternalOutput")

    with tile.TileContext(nc) as tc:
        with (
            tc.tile_pool(name="wb", bufs=1) as wb,
            tc.tile_pool(name="act", bufs=2) as ap,
            tc.tile_pool(name="ps", bufs=8, space="PSUM") as psp,
            tc.tile_pool(name="dram", bufs=2, space="DRAM") as drp,
        ):
            def load(pool, dram, tag):
                t = pool.tile(list(dram.shape), F32, tag=tag)
                nc.sync.dma_start(t[:], dram[:])
                return t

            # persistent consts
            ident = load(wb, ident_d, "ident")
            gsT = load(wb, gsT_d, "gsT")
            pj = load(wb, pj_d, "pj")
            padm = load(wb, padm_d, "padm")
            mesd = load(wb, mesd_d, "mesd")
            agcnT = load(wb, agcnT_d, "agcnT")
            s1l1 = load(wb, s1l1_d, "s1l1")
            b_s1l1 = load(wb, b_s1l1_d, "b_s1l1")
            w1T = load(wb, w1T_d, "w1T")
            b1 = load(wb, b1_d, "b1")
            w2T = load(wb, w2T_d, "w2T")
            b2 = load(wb, b2_d, "b2")
            w3T = load(wb, w3T_d, "w3T")
            b3 = load(wb, b3_d, "b3")
            fc0b = load(wb, fc0b_d, "fc0b")
            fc1b = load(wb, fc1b_d, "fc1b")
            fc2T = load(wb, fc2T_d, "fc2T")
            fc2b = load(wb, fc2b_d, "fc2b")
            fc3T = load(wb, fc3T_d, "fc3T")
            fc3b = load(wb, fc3b_d, "fc3b")
            fcgb = load(wb, fcgb_d, "fcgb")
            fu_i0 = load(wb, fu_i0_d, "fu_i0")
            fu_i1 = load(wb, fu_i1_d, "fu_i1")
            fu_s = load(wb, fu_s_d, "fu_s")
            fu_g = load(wb, fu_g_d, "fu_g")
            fub = load(wb, fub_d, "fub")
            headT = load(wb, headT_d, "headT")
            headb = load(wb, headb_d, "headb")
            sb = {k: load(wb, d, k) for k, d in shard_b.items()}
            g0_base = load(wb, g0_d, "g0")
            st_base = load(wb, st_d, "st")
            zgate = {}
            if repeat > 1:
                for nm, p in [("z19", 19), ("z4", 4), ("z32", 32)]:
                    zt = wb.tile([2, p], F32, tag=nm)
                    nc.vector.memset(zt[:], 0.0)
                    zgate[nm] = zt

            o_sb = None

            def mm8(psum, w_sb, xt, n_out=FS):
                for j in range(8):
                    nc.tensor.matmul(
                        psum[:], w_sb[:, ts(j, n_out)], xt[:, j, :],
                        start=(j == 0), stop=(j == 7),
                    )

            def ag_x(z, tag):
                """z [125,19] slice -> AllGather -> xt [125,8,19] (full, feature-major)."""
                bi = drp.tile([FS, 19], F32, tag=f"agi_{tag}")
                bo = drp.tile([1000, 19], F32, tag=f"ago_{tag}")
                nc.sync.dma_start(bi[:], z[:])
                nc.gpsimd.collective_compute(
                    "AllGather", ALU.bypass, replica_groups=RG,
                    ins=[bi.opt()], outs=[bo.opt()],
                )
                xt = ap.tile([FS, 8, 19], F32, tag="xfull")
                nc.sync.dma_start(xt[:], bo[:].rearrange("(j p) s -> p j s", j=8))
                return xt, bo

            def sage(xt, wl_sb, wr_sb, b_sb):
                ps_y = psp.tile([FS, 19], F32, tag="ps")
                mm8(ps_y, wl_sb, xt)
                agg = ap.tile([FS, 19], F32, tag="agg")
                nc.vector.memset(agg[:, 0:1], 0.0)
                nc.vector.tensor_copy(agg[:, 1:19], ps_y[:, 0:18])
                nc.vector.tensor_add(agg[:, 0:17], agg[:, 0:17], ps_y[:, 1:18])
                nc.vector.tensor_add(agg[:, 17:18], agg[:, 17:18], ps_y[:, 18:19])
                ps_o = psp.tile([FS, 19], F32, tag="ps")
                mm8(ps_o, wr_sb, xt)
                nc.vector.tensor_add(ps_o[:], ps_o[:], agg[:])
                z = ap.tile([FS, 19], F32, tag="z")
                nc.scalar.activation(z[:], ps_o[:], AF.Relu, bias=b_sb[:])
                return z

            def gat(xt, w_sb, as_sb, ad_sb, b_sb, tag):
                ps_hh = psp.tile([FS, 19], F32, tag="ps")
                mm8(ps_hh, w_sb, xt)
                hh = ap.tile([FS, 19], F32, tag="hh")
                nc.scalar.activation(hh[:], ps_hh[:], AF.Copy)
                ps_ds = psp.tile([1, 19], F32, tag="ps")
                nc.tensor.matmul(ps_ds[:], as_sb[:], hh[:], start=True, stop=True)
                ps_dd = psp.tile([1, 19], F32, tag="ps")
                nc.tensor.matmul(ps_dd[:], ad_sb[:], hh[:], start=True, stop=True)
                dots = ap.tile([1, 38], F32, tag="dots")
                nc.scalar.activation(dots[:, 0:19], ps_ds[:], AF.Copy)
                nc.scalar.activation(dots[:, 19:38], ps_dd[:], AF.Copy)
                bi = drp.tile([1, 38], F32, tag=f"dotsi_{tag}")
                bo = drp.tile([2 * NCORE, 19], F32, tag=f"dotso_{tag}")
                nc.sync.dma_start(bi[:], dots[:])
                nc.gpsimd.collective_compute(
                    "AllGather", ALU.bypass, replica_groups=RG,
                    ins=[bi.opt()], outs=[bo.opt()],
                )
                agres = ap.tile([2 * NCORE, 19], F32, tag="agres")
                nc.sync.dma_start(agres[:], bo[:])
                ps_esd = psp.tile([19, 2], F32, tag="ps")
                nc.tensor.matmul(ps_esd[:], agres[:], mesd[:], start=True, stop=True)
                esd = ap.tile([19, 2], F32, tag="esd")
                nc.scalar.activation(esd[:], ps_esd[:], AF.Copy)
                ps_eg = psp.tile([19, DEG], F32, tag="ps")
                for j in range(DEG):
                    nc.tensor.matmul(
                        ps_eg[:, j:j + 1], gsT[:, ts(j, 19)], esd[:, 0:1],
                        start=True, stop=True,
                    )
                # leaky_relu(es + ed, 0.2) + pad mask
                eg0 = ap.tile([19, DEG], F32, tag="eg0")
                nc.vector.tensor_scalar_add(eg0[:], ps_eg[:], esd[:, 1:2])
                eg2 = ap.tile([19, DEG], F32, tag="eg2")
                nc.vector.tensor_scalar_mul(eg2[:], eg0[:], 0.2)
                nc.vector.tensor_max(eg0[:], eg0[:], eg2[:])
                nc.vector.tensor_add(eg0[:], eg0[:], padm[:])
                negm = ap.tile([19, 1], F32, tag="negm")
                nc.vector.tensor_reduce(negm[:], eg0[:], axis=AX.X, op=ALU.max, negate=True)
                ex = ap.tile([19, DEG], F32, tag="ex")
                nc.scalar.activation(ex[:], eg0[:], AF.Exp, bias=negm[:])
                den = ap.tile([19, 1], F32, tag="den")
                nc.vector.reduce_sum(den[:], ex[:], axis=AX.X)
                rden = ap.tile([19, 1], F32, tag="rden")
                nc.vector.reciprocal(rden[:], den[:])
                alph = ap.tile([19, DEG], F32, tag="alph")
                nc.vector.tensor_scalar_mul(alph[:], ex[:], rden[:])
                aa = ap.tile([19, 19], F32, tag="aa")
                tmp = ap.tile([19, 19], F32, tag="aatmp")
                nc.vector.tensor_scalar_mul(aa[:], pj[:, 0:19], alph[:, 0:1])
                for j in (1, 2):
                    nc.vector.tensor_scalar_mul(tmp[:], pj[:, ts(j, 19)], alph[:, j:j + 1])
                    nc.vector.tensor_add(aa[:], aa[:], tmp[:])
                ps_aat = psp.tile([19, 19], F32, tag="ps")
                nc.tensor.transpose(ps_aat[:], aa[:], ident[0:19, 0:19])
                aat = ap.tile([19, 19], F32, tag="aat")
                nc.scalar.activation(aat[:], ps_aat[:], AF.Copy)
                ps_hht = psp.tile([19, FS], F32, tag="ps")
                nc.tensor.transpose(ps_hht[:], hh[:], ident[0:FS, 0:FS])
                hht = ap.tile([19, FS], F32, tag="hht")
                nc.scalar.activation(hht[:], ps_hht[:], AF.Copy)
                ps_msg = psp.tile([FS, 19], F32, tag="ps")
                nc.tensor.matmul(ps_msg[:], hht[:], aat[:], start=True, stop=True)
                z = ap.tile([FS, 19], F32, tag="z")
                nc.scalar.activation(z[:], ps_msg[:], AF.Relu, bias=b_sb[:])
                return z

            for it in range(repeat):
                # per-iteration streamed weights
                sw = {k: load(wb, d, k) for k, d in shard_w.items()}
                fc0A = load(wb, fc0A_d, "fc0A")
                fc0B = load(wb, fc0B_d, "fc0B")
                fc1A = load(wb, fc1A_d, "fc1A")
                fc1B = load(wb, fc1B_d, "fc1B")
                fcgT = load(wb, fcgT_d, "fcgT")
                p1 = load(wb, p1_d, "p1")

                # iteration gating (serialize iterations for latency timing)
                if it == 0 or repeat == 1:
                    g0, st = g0_base, st_base
                    c3gate = None
                else:
                    ps_z19 = psp.tile([19, 1], F32, tag="ps")
                    nc.tensor.matmul(ps_z19[:], zgate["z19"][:], o_sb[:], start=True, stop=True)
                    g0 = ap.tile([19, 1], F32, tag="g0it")
                    nc.vector.tensor_add(g0[:], g0_base[:], ps_z19[:])
                    ps_z4 = psp.tile([4, 1], F32, tag="ps")
                    nc.tensor.matmul(ps_z4[:], zgate["z4"][:], o_sb[:], start=True, stop=True)
                    st = ap.tile([4, 1], F32, tag="stit")
                    nc.vector.tensor_add(st[:], st_base[:], ps_z4[:])
                    ps_z32 = psp.tile([32, 1], F32, tag="ps")
                    nc.tensor.matmul(ps_z32[:], zgate["z32"][:], o_sb[:], start=True, stop=True)
                    c3gate = ap.tile([32, 1], F32, tag="c3gate")
                    nc.scalar.activation(c3gate[:], ps_z32[:], AF.Copy)

                # ---------------- graph chain ----------------
                # s1l1 (replicated): g_row/agg_row then 1000-wide outer products
                ps_gr = psp.tile([1, 19], F32, tag="ps")
                nc.tensor.transpose(ps_gr[:], g0[:], ident[0:19, 0:19])
                grow = ap.tile([1, 19], F32, tag="grow")
                nc.scalar.activation(grow[:], ps_gr[:], AF.Copy)
                arow = ap.tile([1, 19], F32, tag="arow")
                nc.vector.memset(arow[:, 0:1], 0.0)
                nc.vector.tensor_copy(arow[:, 1:19], grow[:, 0:18])
                nc.vector.tensor_add(arow[:, 0:17], arow[:, 0:17], grow[:, 1:18])
                nc.vector.tensor_add(arow[:, 17:18], arow[:, 17:18], grow[:, 18:19])
                x1 = ap.tile([FS, 8, 19], F32, tag="xfull")
                for j in range(8):
                    ps1 = psp.tile([FS, 19], F32, tag="ps")
                    nc.tensor.matmul(ps1[:], s1l1[:, ts(j, FS)], arow[:], start=True, stop=False)
                    nc.tensor.matmul(ps1[:], s1l1[:, 1000 + FS * j:1000 + FS * (j + 1)],
                                     grow[:], start=False, stop=True)
                    nc.scalar.activation(x1[:, j, :], ps1[:], AF.Relu, bias=b_s1l1[:, j:j + 1])

                z2 = sage(x1, sw["w_s1l2_l"], sw["w_s1l2_r"], sb["b_s1l2"])
                x2, bo2 = ag_x(z2, "x2")
                z3 = gat(x2, sw["w_gat2"], sb["as2"], sb["ad2"], sb["b_gat2"], "g2")
                x3, bo3 = ag_x(z3, "x3")
                z4 = sage(x3, sw["w_s3l1_l"], sw["w_s3l1_r"], sb["b_s3l1"])
                x4, bo4 = ag_x(z4, "x4")
                z5 = sage(x4, sw["w_s3l2_l"], sw["w_s3l2_r"], sb["b_s3l2"])
                x5, bo5 = ag_x(z5, "x5")
                z6 = gat(x5, sw["w_gat4"], sb["as4"], sb["ad4"], sb["b_gat4"], "g4")
                x6, bo6 = ag_x(z6, "x6")

                # gcn5 (sharded) + mean over nodes
                ps_hh = psp.tile([FS, 19], F32, tag="ps")
                mm8(ps_hh, sw["w_gcn5"], x6)
                hh5 = ap.tile([FS, 19], F32, tag="hh")
                nc.scalar.activation(hh5[:], ps_hh[:], AF.Copy)
                ps_hht = psp.tile([19, FS], F32, tag="ps")
                nc.tensor.transpose(ps_hht[:], hh5[:], ident[0:FS, 0:FS])
                hht5 = ap.tile([19, FS], F32, tag="hht")
                nc.scalar.activation(hht5[:], ps_hht[:], AF.Copy)
                ps_g = psp.tile([FS, 19], F32, tag="ps")
                nc.tensor.matmul(ps_g[:], hht5[:], agcnT[:], start=True, stop=True)
                gs = ap.tile([FS, 1], F32, tag="gs")
                nc.vector.reduce_sum(gs[:], ps_g[:], axis=AX.X)
                gm = ap.tile([FS, 1], F32, tag="gm")
                nc.scalar.activation(gm[:], gs[:], AF.Identity, bias=sb["b_gcn5"][:],
                                     scale=1.0 / 19.0)
                bgi = drp.tile([FS, 1], F32, tag="bgi")
                bgo = drp.tile([1000, 1], F32, tag="bgo")
                nc.sync.dma_start(bgi[:], gm[:])
                nc.gpsimd.collective_compute(
                    "AllGather", ALU.bypass, replica_groups=RG,
                    ins=[bgi.opt()], outs=[bgo.opt()],
                )
                gt8 = ap.tile([8, FS], F32, tag="gt8")
                nc.sync.dma_start(gt8[:], bgo[:].rearrange("(j p) s -> j (p s)", j=8))
                ps_gt = psp.tile([FS, 8], F32, tag="ps")
                nc.tensor.transpose(ps_gt[:], gt8[:], ident[0:8, 0:8])
                gt = ap.tile([FS, 8], F32, tag="gt")
                nc.scalar.activation(gt[:], ps_gt[:], AF.Copy)
                ps_g100 = psp.tile([100, 1], F32, tag="ps")
                for j in range(8):
                    nc.tensor.matmul(ps_g100[:], fcgT[:, ts(j, 100)], gt[:, j:j + 1],
                                     start=(j == 0), stop=(j == 7))
                g100 = ap.tile([100, 1], F32, tag="g100")
                nc.scalar.activation(g100[:], ps_g100[:], AF.Relu, bias=fcgb[:])

                # ---------------- CNN branch ----------------
                conv1 = ap.tile([32, 3721], F32, tag="conv1")
                for c in range(8):
                    n0 = 512 * c
                    n = min(512, 3721 - n0)
                    psc = psp.tile([32, n], F32, tag="ps")
                    nc.tensor.matmul(psc[:], w1T[:], p1[:, n0:n0 + n], start=True, stop=True)
                    nc.scalar.activation(conv1[:, n0:n0 + n], psc[:], AF.Relu, bias=b1[:])
                v1 = conv1[:].rearrange("p (y x) -> p y x", y=61)
                conv2 = ap.tile([32, 29, 29], F32, tag="conv2")
                for half, (y0, ny) in enumerate([(0, 15), (15, 14)]):
                    psc = psp.tile([32, ny, 29], F32, tag="ps")
                    for off in range(25):
                        ky, kx = off // 5, off % 5
                        r0 = 2 * y0 + ky
                        rhs = v1[:, r0:r0 + 2 * ny - 1:2, kx:kx + 57:2]
                        nc.tensor.matmul(psc[:], w2T[:, ts(off, 32)], rhs,
                                         start=(off == 0), stop=(off == 24))
                    nc.scalar.activation(conv2[:, y0:y0 + ny, :], psc[:], AF.Relu, bias=b2[:])
                pad3 = ap.tile([32, 961], F32, tag="pad3")
                nc.vector.memset(pad3[:], 0.0)
                v3 = pad3[:].rearrange("p (y x) -> p y x", y=31)
                nc.vector.tensor_copy(v3[:, 1:30, 1:30], conv2[:])
                ps3 = psp.tile([32, 14, 14], F32, tag="ps")
                for off in range(25):
                    ky, kx = off // 5, off % 5
                    rhs = v3[:, ky:ky + 27:2, kx:kx + 27:2]
                    nc.tensor.matmul(ps3[:], w3T[:, ts(off, 32)], rhs,
                                     start=(off == 0), stop=(off == 24))
                conv3 = ap.tile([32, 196], F32, tag="conv3")
                if c3gate is None:
                    nc.scalar.activation(conv3[:], ps3[:].rearrange("p a b -> p (a b)"),
                                         AF.Identity, bias=b3[:])
                else:
                    c3b = ap.tile([32, 1], F32, tag="c3b")
                    nc.vector.tensor_add(c3b[:], b3[:], c3gate[:])
                    nc.scalar.activation(conv3[:], ps3[:].rearrange("p a b -> p (a b)"),
                                         AF.Identity, bias=c3b[:])
                # fc0 (sharded 128 outputs/core): transpose conv3 into K-major chunks
                ps_hA = psp.tile([128, 32], F32, tag="ps")
                nc.tensor.transpose(ps_hA[:], conv3[:, 0:128], ident[0:32, 0:32])
                hA = ap.tile([128, 32], F32, tag="hA")
                nc.scalar.activation(hA[:], ps_hA[:], AF.Copy)
                ps_hB = psp.tile([68, 32], F32, tag="ps")
                nc.tensor.transpose(ps_hB[:], conv3[:, 128:196], ident[0:32, 0:32])
                hB = ap.tile([68, 32], F32, tag="hB")
                nc.scalar.activation(hB[:], ps_hB[:], AF.Copy)
                ps_h = psp.tile([128, 1], F32, tag="ps")
                for c in range(32):
                    nc.tensor.matmul(ps_h[:], fc0A[:, ts(c, 128)], hA[:, c:c + 1],
                                     start=(c == 0), stop=False)
                for c in range(32):
                    nc.tensor.matmul(ps_h[:], fc0B[:, ts(c, 128)], hB[:, c:c + 1],
                                     start=False, stop=(c == 31))
                hk = ap.tile([128, 1], F32, tag="hk")
                nc.scalar.activation(hk[:], ps_h[:], AF.Relu, bias=fc0b[:])
                bhi = drp.tile([128, 1], F32, tag="bhi")
                bho = drp.tile([1024, 1], F32, tag="bho")
                nc.sync.dma_start(bhi[:], hk[:])
                nc.gpsimd.collective_compute(
                    "AllGather", ALU.bypass, replica_groups=RG,
                    ins=[bhi.opt()], outs=[bho.opt()],
                )
                h8 = ap.tile([8, 128], F32, tag="h8")
                nc.sync.dma_start(h8[:], bho[:].rearrange("(j p) s -> j (p s)", j=8))
                ps_hf = psp.tile([128, 8], F32, tag="ps")
                nc.tensor.transpose(ps_hf[:], h8[:], ident[0:8, 0:8])
                hf = ap.tile([128, 8], F32, tag="hf")
                nc.scalar.activation(hf[:], ps_hf[:], AF.Copy)
                img = ap.tile([128, 2], F32, tag="img")
                for half, fw in enumerate([fc1A, fc1B]):
                    ps_i = psp.tile([128, 1], F32, tag="ps")
                    for j in range(8):
                        nc.tensor.matmul(ps_i[:], fw[:, ts(j, 128)], hf[:, j:j + 1],
                                         start=(j == 0), stop=(j == 7))
                    nc.scalar.activation(img[:, half:half + 1], ps_i[:], AF.Relu,
                                         bias=fc1b[:, half:half + 1])

                # ---------------- state branch ----------------
                ps_s1 = psp.tile([64, 1], F32, tag="ps")
                nc.tensor.matmul(ps_s1[:], fc2T[:], st[:], start=True, stop=True)
                s1 = ap.tile([64, 1], F32, tag="s1")
                nc.scalar.activation(s1[:], ps_s1[:], AF.Relu, bias=fc2b[:])
                ps_s2 = psp.tile([64, 1], F32, tag="ps")
                nc.tensor.matmul(ps_s2[:], fc3T[:], s1[:], start=True, stop=True)
                s2 = ap.tile([64, 1], F32, tag="s2")
                nc.scalar.activation(s2[:], ps_s2[:], AF.Relu, bias=fc3b[:])

                # ---------------- fusion + heads ----------------
                fsb = ap.tile([128, 2], F32, tag="fsb")
                for half in range(2):
                    ps_f = psp.tile([128, 1], F32, tag="ps")
                    nc.tensor.matmul(ps_f[:], fu_i0[:, ts(half, 128)], img[:, 0:1],
                                     start=True, stop=False)
                    nc.tensor.matmul(ps_f[:], fu_i1[:, ts(half, 128)], img[:, 1:2],
                                     start=False, stop=False)
                    nc.tensor.matmul(ps_f[:], fu_s[:, ts(half, 128)], s2[:],
                                     start=False, stop=False)
                    nc.tensor.matmul(ps_f[:], fu_g[:, ts(half, 128)], g100[:],
                                     start=False, stop=True)
                    nc.scalar.activation(fsb[:, half:half + 1], ps_f[:], AF.Relu,
                                         bias=fub[:, half:half + 1])
                ps_o = psp.tile([2, 1], F32, tag="ps")
                for j in range(2):
                    nc.tensor.matmul(ps_o[:], headT[:, ts(j, 2)], fsb[:, j:j + 1],
                                     start=(j == 0), stop=(j == 1))
                o_sb = ap.tile([2, 1], F32, tag="o_sb")
                nc.scalar.activation(o_sb[:], ps_o[:], AF.Identity, bias=headb[:])
                nc.sync.dma_start(out_t[:], o_sb[:])

                if debug and it == repeat - 1:
                    nc.sync.dma_start(dbg_t["dbg_xf2"][:], bo2[:])
                    nc.sync.dma_start(dbg_t["dbg_xf3"][:], bo3[:])
                    nc.sync.dma_start(dbg_t["dbg_xf4"][:], bo4[:])
                    nc.sync.dma_start(dbg_t["dbg_xf5"][:], bo5[:])
                    nc.sync.dma_start(dbg_t["dbg_xf6"][:], bo6[:])
                    nc.sync.dma_start(
                        dbg_t["dbg_xf1"][:].rearrange("(j p) s -> p j s", j=8), x1[:])
                    nc.sync.dma_start(dbg_t["dbg_gmf"][:], bgo[:])
                    nc.sync.dma_start(dbg_t["dbg_hf"][:], bho[:])
                    nc.sync.dma_start(dbg_t["dbg_img"][:], img[:])
                    nc.sync.dma_start(dbg_t["dbg_s2"][:], s2[:])
                    nc.sync.dma_start(dbg_t["dbg_g100"][:], g100[:])
                    nc.sync.dma_start(dbg_t["dbg_conv1"][:], conv1[:])
                    nc.sync.dma_start(dbg_t["dbg_conv2"][:],
                                      conv2[:].rearrange("p a b -> p (a b)"))
                    nc.sync.dma_start(dbg_t["dbg_conv3"][:], conv3[:])
                    nc.sync.dma_start(dbg_t["dbg_f"][:], fsb[:])

    nc.compile()
    return nc, names


def host_prep(x, state, x_graph, params):
    """Build per-core input maps (all arrays float32, contiguous)."""
    p = {k: np.asarray(v, np.float32) for k, v in params.items()}
    x = np.asarray(x, np.float32)
    state = np.asarray(state, np.float32)
    x_graph = np.asarray(x_graph, np.float32)

    gsT, pjm, padm, agcnT, mesd = _graph_consts()

    # conv1 im2col (with pad=1, stride 2) — pure reindexing of the input image
    xpad = np.zeros((126, 126), np.float32)
    xpad[1:125, 1:125] = x
    p1 = np.empty((25, 3721), np.float32)
    for ky in range(5):
        for kx in range(5):
            p1[ky * 5 + kx] = xpad[ky:ky + 122:2, kx:kx + 122:2].reshape(-1)

    w1T = np.ascontiguousarray(p["conv1_w"].reshape(32, 25).T)
    w2T = np.ascontiguousarray(p["conv2_w"].transpose(1, 2, 3, 0).reshape(32, 25 * 32))
    w3T = np.ascontiguousarray(p["conv3_w"].transpose(1, 2, 3, 0).reshape(32, 25 * 32))

    def fc_pack(w, half):  # w [256,1024] -> [128, 1024] lhsT image for output half
        t = w[128 * half:128 * (half + 1), :]
        return np.ascontiguousarray(
            t.reshape(128, 8, 128).transpose(2, 1, 0).reshape(128, 1024))

    fc1A = fc_pack(p["fc1_w"], 0)
    fc1B = fc_pack(p["fc1_w"], 1)
    fc1b = np.ascontiguousarray(p["fc1_b"].reshape(2, 128).T)

    fcgT = _pack_kt(p["fc_graph_w"])  # [100,1000] -> [125, 800]

    fw = p["fc_fusion_w"]  # [256, 420]

    def fu_pack(cols):
        t = fw[:, cols]  # [256, K]
        return np.ascontiguousarray(t.T.reshape(t.shape[1], 256))

    fu_i0 = fu_pack(slice(0, 128))
    fu_i1 = fu_pack(slice(128, 256))
    fu_s = fu_pack(slice(256, 320))
    fu_g = fu_pack(slice(320, 420))
    fub = np.ascontiguousarray(p["fc_fusion_b"].reshape(2, 128).T)

    vw = np.stack([p["fc_value_w"][0], p["fc_advantage_w"][0]])  # [2, 256]
    headT = np.ascontiguousarray(vw.T.reshape(2, 128, 2).transpose(1, 0, 2).reshape(128, 4))
    # headT[p, 2j+t] = vw[t, 128j + p]
    headb = np.array([[p["fc_value_b"][0]], [p["fc_advantage_b"][0]]], np.float32)

    rep = {
        "s1l1_wlr": np.concatenate([p["s1l1_wl"][:, 0], p["s1l1_wr"][:, 0]])[None, :],
        "b_s1l1": np.ascontiguousarray(p["s1l1_bl"].reshape(8, FS).T),
        "p1": p1, "w1T": w1T, "b1": p["conv1_b"][:, None],
        "w2T": w2T, "b2": p["conv2_b"][:, None],
        "w3T": w3T, "b3": p["conv3_b"][:, None],
        "fc1A": fc1A, "fc1B": fc1B, "fc1b": fc1b,
        "fc2T": np.ascontiguousarray(p["fc2_w"].T), "fc2b": p["fc2_b"][:, None],
        "fc3T": np.ascontiguousarray(p["fc3_w"].T), "fc3b": p["fc3_b"][:, None],
        "fcgT": fcgT, "fcgb": p["fc_graph_b"][:, None],
        "fu_i0": fu_i0, "fu_i1": fu_i1, "fu_s": fu_s, "fu_g": fu_g, "fub": fub,
        "headT": headT, "headb": headb,
        "ident": np.eye(128, dtype=np.float32),
        "gsT": gsT, "pj": pjm, "padm": padm, "mesd": mesd, "agcnT": agcnT,
        "g0": x_graph[:, None], "st": state[:, None],
    }
    rep = {k: np.ascontiguousarray(v, dtype=np.float32) for k, v in rep.items()}

    in_maps = []
    for k in range(NCORE):
        fs = slice(FS * k, FS * (k + 1))
        os_ = slice(OS * k, OS * (k + 1))
        m = dict(rep)
        m["w_s1l2_l"] = _pack_kt(p["s1l2_wl"][fs])
        m["w_s1l2_r"] = _pack_kt(p["s1l2_wr"][fs])
        m["w_gat2"] = _pack_kt(p["gat2_w"][fs])
        m["w_s3l1_l"] = _pack_kt(p["s3l1_wl"][fs])
        m["w_s3l1_r"] = _pack_kt(p["s3l1_wr"][fs])
        m["w_s3l2_l"] = _pack_kt(p["s3l2_wl"][fs])
        m["w_s3l2_r"] = _pack_kt(p["s3l2_wr"][fs])
        m["w_gat4"] = _pack_kt(p["gat4_w"][fs])
        m["w_gcn5"] = _pack_kt(p["gcn5_w"][fs])
        m["b_s1l2"] = p["s1l2_bl"][fs, None]
        m["b_gat2"] = p["gat2_b"][fs, None]
        m["b_s3l1"] = p["s3l1_bl"][fs, None]
        m["b_s3l2"] = p["s3l2_bl"][fs, None]
        m["b_gat4"] = p["gat4_b"][fs, None]
        m["b_gcn5"] = p["gcn5_b"][fs, None]
        m["as2"] = p["gat2_as"][fs, None]
        m["ad2"] = p["gat2_ad"][fs, None]
        m["as4"] = p["gat4_as"][fs, None]
        m["ad4"] = p["gat4_ad"][fs, None]
        wk = p["fc0_w"][os_].reshape(128, 32, 196)
        m["fc0A"] = np.ascontiguousarray(wk[:, :, :128].transpose(2, 1, 0).reshape(128, 32 * 128))
        m["fc0B"] = np.ascontiguousarray(wk[:, :, 128:].transpose(2, 1, 0).reshape(68, 32 * 128))
        m["fc0b"] = p["fc0_b"][os_, None]
        m = {kk: np.ascontiguousarray(vv, dtype=np.float32) for kk, vv in m.items()}
        in_maps.append(m)
    return in_maps


_CACHE = {}


def _get_program(repeat=1, debug=False):
    key = (repeat, debug)
    if key not in _CACHE:
        _CACHE[key] = build_program(repeat=repeat, debug=debug)
    return _CACHE[key]


def kernel(x, state, x_graph, params):
    nc, _ = _get_program(repeat=1, debug=False)
    in_maps = host_prep(x, state, x_graph, params)
    res = bass_utils.run_bass_kernel_spmd(nc, in_maps, core_ids=list(range(NCORE)))
    out = res.results[0]["out"]
    return np.float32(out[0, 0]), np.float32(out[1, 0])
